# revision 27
# baseline (speedup 1.0000x reference)
"""GQA attention kernel for Trainium2, sharded over 8 NeuronCores.

Sharding: tensor-parallel over heads. Core c owns kv-head c and q-heads
4c..4c+3 (rows 256c:256c+256 of Wq, rows 64c:64c+64 of Wk/Wv) and columns
256c:256c+256 of Wo. Each core computes a full-shape partial of the output
(o_proj column-parallel); the host sums the 8 partials (the all-reduce)
and adds bo.

Per-core kernel layout choices:
- hidden_states is passed transposed [H, B*S] so QKV projections contract
  over the partition dim with contiguous DMA.
- Q,K,V are produced transposed ([feature, token]) directly from the PE.
- Scores are computed transposed, S^T[t, q] = K_d,t^T . Q_d,q, so the
  softmax mask+scale fold into the exp activation (mask is per-partition),
  and a ones-column appended to V yields softmax denominators as row 64 of
  the context matmul output.
- All matmuls use float32r (full-rate fp32 on TRN2 when N >= 256).
"""

import os
import sys

for _p in ("/opt/trn_rl_repo",):
    if _p not in sys.path and os.path.isdir(_p):
        sys.path.insert(0, _p)

import numpy as np

import concourse.bass as bass
import concourse.bacc as bacc
import concourse.tile as tile
from concourse import mybir
from concourse import bass_utils

F32 = mybir.dt.float32
F32R = mybir.dt.float32r
AF = mybir.ActivationFunctionType

B = 2
S = 2048
H = 2048
D = 64
N_CORES = 8
QH_PER_CORE = 4          # q-heads per core
QF = QH_PER_CORE * D     # 256 q features per core
TOK = B * S              # 4096
SCALE = 1.0 / np.sqrt(D)  # 0.125

_CACHE = {}


def _build_program():
    nc = bacc.Bacc("TRN2", target_bir_lowering=False, debug=False)

    hsT = nc.dram_tensor("hsT", [H, TOK], F32R, kind="ExternalInput").ap()
    wqkvT = nc.dram_tensor("wqkvT", [H, 384], F32R, kind="ExternalInput").ap()
    woT = nc.dram_tensor("woT", [QF, H], F32R, kind="ExternalInput").ap()
    bqkv = nc.dram_tensor("bqkv", [128, 3], F32, kind="ExternalInput").ap()
    maskp = nc.dram_tensor("maskp", [128, B, S // 128], F32, kind="ExternalInput").ap()
    eye = nc.dram_tensor("eye", [128, 64], F32R, kind="ExternalInput").ap()
    out = nc.dram_tensor("out", [B, S, H], F32, kind="ExternalOutput").ap()

    with tile.TileContext(nc) as tc:
        with tc.tile_pool(name="const", bufs=1) as cp:
            w_qkv = cp.tile([128, 16, 384], F32R)     # (p, h_tile, feature)
            nc.sync.dma_start(out=w_qkv, in_=wqkvT.rearrange("(t p) f -> p t f", p=128))
            w_o = cp.tile([128, 2, H], F32R)          # (p, f_tile, e)
            nc.sync.dma_start(out=w_o, in_=woT.rearrange("(t p) e -> p t e", p=128))
            bqkv_sb = cp.tile([128, 3], F32)
            nc.sync.dma_start(out=bqkv_sb, in_=bqkv)
            mask_sb = cp.tile([128, B, S // 128], F32)
            nc.sync.dma_start(out=mask_sb, in_=maskp)
            eye_sb = cp.tile([128, 64], F32R)
            nc.sync.dma_start(out=eye_sb, in_=eye)

            # Engine wait budgets are tiny (1 sync-wait per instruction for
            # PE/ACT structs). Warm each consumer engine's vector clock on the
            # small const DMAs so real instructions never need a second wait.
            scratch = cp.tile([128, 1], F32)
            nc.scalar.copy(out=scratch, in_=bqkv_sb[:, 0:1])
            nc.scalar.copy(out=scratch, in_=mask_sb[:, 0, 0:1])

            # Q^T, K^T, V^T resident in SBUF: qkvT[0] = q feats 0:128,
            # qkvT[1] = q feats 128:256, qkvT[2] = [K (0:64) | V (64:128)].
            qkvT = [cp.tile([128, TOK], F32R, name=f"qkvT{i}") for i in range(3)]
            # V transposed back to [t, d] + ones column, per 128-token tile.
            vones = cp.tile([128, B * 16, 65], F32R)
            # K^T replicated in both partition halves so each q-head's scores
            # matmul finds K at its own base partition (PE alignment rule).
            k2 = cp.tile([128, TOK], F32R)

            # ---- QKV projections ----
            hsT_tiled = hsT.rearrange("(t p) n -> p t n", p=128)
            with tc.tile_pool(name="proj_sb", bufs=2) as psb, \
                 tc.tile_pool(name="proj_ps", bufs=3, space="PSUM") as pps:
                CK = 256
                # fp32r matmuls encode a single sync-wait slot. A dummy [1,1]
                # matmul "spends" one DMA wait on the PE clock so the first
                # real matmul of each chunk only needs its remaining wait.
                dps = pps.tile([1, 1], F32, tag="dummy", bufs=1)
                nc.tensor.matmul(dps, w_o[:, 0, 0:1].bitcast(F32), w_o[:, 0, 0:1].bitcast(F32),
                                 start=True, stop=True)
                for ck in range(TOK // CK):
                    hstage = psb.tile([128, 16, CK], F32R, tag="hstage")
                    # 16 piece-DMAs: queue round-robin makes slot reuse land
                    # on the same queue (implicit WAW), and each consuming
                    # matmul carries exactly one piece-wait.
                    for ht in range(16):
                        nc.sync.dma_start(
                            out=hstage[:, ht, :],
                            in_=hsT_tiled[:, ht, ck * CK:(ck + 1) * CK])
                    nc.tensor.matmul(dps, hstage[:, 0, 0:1].bitcast(F32), hstage[:, 0, 0:1].bitcast(F32),
                                     start=True, stop=True)
                    for ft in range(3):
                        ps = pps.tile([128, CK], F32, tag="projps", bufs=3)
                        for ht in range(16):
                            nc.tensor.matmul(
                                ps,
                                w_qkv[:, ht, ft * 128:(ft + 1) * 128],
                                hstage[:, ht, :],
                                start=(ht == 0), stop=(ht == 15),
                            )
                        nc.scalar.activation(
                            out=qkvT[ft][:, ck * CK:(ck + 1) * CK], in_=ps,
                            func=AF.Identity, bias=bqkv_sb[:, ft:ft + 1],
                        )
                # ---- build V[t, d] (+ones) from V^T via PE transpose ----
                for bt in range(B * 16):
                    tp = pps.tile([128, 64], F32R, tag="vtrans", bufs=2)
                    nc.tensor.transpose(
                        tp, in_=qkvT[2][64:128, bt * 128:(bt + 1) * 128],
                        identity=eye_sb[64:128, :])
                    nc.scalar.copy(out=vones[:, bt, 0:64], in_=tp)
                # ones column via ACT (keeps vones single-writer-proc: ACT
                # only), computed as 0*mask + 1 from a known-finite input.
                nc.scalar.activation(
                    out=vones[:, :, 64:65],
                    in_=mask_sb.rearrange("p b t -> p (b t)"),
                    func=AF.Identity, bias=1.0, scale=0.0)
                nc.sync.dma_start(out=k2[0:64, :], in_=qkvT[2][0:64, :])
                nc.sync.dma_start(out=k2[64:128, :], in_=qkvT[2][0:64, :])
                nc.tensor.matmul(dps, k2[0:64, 0:1].bitcast(F32), k2[0:64, 0:1].bitcast(F32),
                                 start=True, stop=True)
                nc.tensor.matmul(dps, k2[64:128, 0:1].bitcast(F32), k2[64:128, 0:1].bitcast(F32),
                                 start=True, stop=True)

            # ---- attention + o_proj ----
            with tc.tile_pool(name="att_sb", bufs=3) as asb, \
                 tc.tile_pool(name="drain_sb", bufs=3) as dsb, \
                 tc.tile_pool(name="ctxT_sb", bufs=2) as csb, \
                 tc.tile_pool(name="scores_ps", bufs=2, space="PSUM") as sps, \
                 tc.tile_pool(name="ctx_ps", bufs=2, space="PSUM") as xps, \
                 tc.tile_pool(name="o_ps", bufs=2, space="PSUM") as ops_pool:
                for b in range(B):
                    for qh in range(2):          # 1024-token q chunks
                        q0 = b * S + qh * 1024
                        ctxT = [csb.tile([128, 1024], F32R, tag=f"ctxT{ft}",
                                         name=f"ctxT{ft}_{b}_{qh}") for ft in range(2)]
                        # pre-spend the ctxT slot-reuse wait (PE o_proj
                        # release) on DVE before the first normalize write
                        for ft in range(2):
                            nc.vector.memset(ctxT[ft][0:1, 0:1].bitcast(F32), 0.0)
                        for g in range(QH_PER_CORE):
                            qt = qkvT[g // 2]
                            qp = (g % 2) * 64
                            ctx0 = xps.tile([65, 512], F32, tag="ctx")
                            ctx1 = xps.tile([65, 512], F32, tag="ctx")
                            ctxs = (ctx0, ctx1)
                            # wait-carrier: spend the ctx-slot WAR wait (DVE
                            # release) before the real t=0 accumulation start.
                            nc.tensor.matmul(ctx0[0:1, 0:1], w_qkv[:, 0, 0:1].bitcast(F32),
                                             w_qkv[:, 0, 0:1].bitcast(F32), start=True, stop=True)
                            nc.tensor.matmul(ctx1[0:1, 0:1], w_qkv[:, 0, 0:1].bitcast(F32),
                                             w_qkv[:, 0, 0:1].bitcast(F32), start=True, stop=True)
                            for t in range(16):
                                sc = sps.tile([128, 1024], F32, tag="scores")
                                for qc in range(2):
                                    nc.tensor.matmul(
                                        sc[:, qc * 512:(qc + 1) * 512],
                                        k2[qp:qp + 64, b * S + t * 128:b * S + (t + 1) * 128],
                                        qt[qp:qp + 64, q0 + qc * 512:q0 + (qc + 1) * 512],
                                        start=True, stop=True,
                                    )
                                ex = asb.tile([128, 1024], F32R, tag="expT")
                                nc.scalar.activation(
                                    out=ex, in_=sc, func=AF.Exp,
                                    bias=mask_sb[:, b, t:t + 1], scale=SCALE,
                                )
                                for qc in range(2):
                                    nc.tensor.matmul(
                                        ctxs[qc],
                                        vones[:, b * 16 + t, :],
                                        ex[:, qc * 512:(qc + 1) * 512],
                                        start=(t == 0), stop=(t == 15),
                                    )
                            # drain: copy out of PSUM, normalize by row 64
                            for qc in range(2):
                                cs = dsb.tile([65, 512], F32, tag="ctx_sb")
                                nc.vector.tensor_copy(out=cs, in_=ctxs[qc])
                                rc = dsb.tile([1, 512], F32, tag="recip")
                                nc.vector.reciprocal(out=rc, in_=cs[64:65, :])
                                bc = dsb.tile([64, 512], F32, tag="bcast")
                                nc.gpsimd.partition_broadcast(bc, rc)
                                nc.vector.tensor_mul(
                                    out=ctxT[g // 2][qp:qp + 64, qc * 512:(qc + 1) * 512],
                                    in0=cs[0:64, :], in1=bc,
                                )
                        # o_proj for this (b, qh): out[tok, e] partial
                        for qq in range(8):
                            osb = asb.tile([128, H], F32, tag="osb", name=f"osb_{b}_{qh}_{qq}")
                            # pre-spend the osb slot-reuse wait (out-DMA done)
                            nc.vector.memset(osb[0:1, 0:1], 0.0)
                            for ec in range(4):
                                op = ops_pool.tile([128, 512], F32, tag="ops")
                                for ft in range(2):
                                    nc.tensor.matmul(
                                        op,
                                        ctxT[ft][:, qq * 128:(qq + 1) * 128],
                                        w_o[:, ft, ec * 512:(ec + 1) * 512],
                                        start=(ft == 0), stop=(ft == 1),
                                    )
                                nc.vector.tensor_copy(
                                    out=osb[:, ec * 512:(ec + 1) * 512], in_=op)
                            nc.sync.dma_start(
                                out=out[b, qh * 1024 + qq * 128:qh * 1024 + (qq + 1) * 128, :],
                                in_=osb,
                            )
    nc.compile()
    return nc


def kernel(hidden_states, attention_mask, Wq, bq, Wk, bk, Wv, bv, Wo, bo):
    hidden_states = np.asarray(hidden_states, dtype=np.float32)
    attention_mask = np.asarray(attention_mask, dtype=np.float32)
    Wq = np.asarray(Wq, dtype=np.float32)
    Wk = np.asarray(Wk, dtype=np.float32)
    Wv = np.asarray(Wv, dtype=np.float32)
    Wo = np.asarray(Wo, dtype=np.float32)

    if "nc" not in _CACHE:
        _CACHE["nc"] = _build_program()
    nc = _CACHE["nc"]

    hsT = np.ascontiguousarray(
        hidden_states.reshape(TOK, H).T)                      # [H, B*S]
    maskp = np.ascontiguousarray(
        attention_mask.reshape(B, S // 128, 128).transpose(2, 0, 1))  # [128, B, 16]
    eye = np.zeros((128, 64), dtype=np.float32)
    eye[64:128, :] = np.eye(64, dtype=np.float32)

    in_maps = []
    for c in range(N_CORES):
        wq = Wq[QF * c:QF * (c + 1)]          # [256, H]
        wk = Wk[D * c:D * (c + 1)]            # [64, H]
        wv = Wv[D * c:D * (c + 1)]            # [64, H]
        wqkvT = np.ascontiguousarray(np.concatenate([wq, wk, wv], axis=0).T)  # [H, 384]
        woT = np.ascontiguousarray(Wo[:, QF * c:QF * (c + 1)].T)              # [256, H]
        bqkv = np.ascontiguousarray(
            np.concatenate([bq[QF * c:QF * (c + 1)], bk[D * c:D * (c + 1)],
                            bv[D * c:D * (c + 1)]]).astype(np.float32)
            .reshape(3, 128).T)               # [128, 3]
        in_maps.append({
            "hsT": hsT, "wqkvT": wqkvT, "woT": woT,
            "bqkv": bqkv, "maskp": maskp, "eye": eye,
        })

    _CACHE["last_in_maps"] = in_maps
    res = bass_utils.run_bass_kernel_spmd(nc, in_maps, core_ids=list(range(N_CORES)))
    acc = np.zeros((B, S, H), dtype=np.float32)
    for c in range(N_CORES):
        acc += res.results[c]["out"]
    acc += np.asarray(bo, dtype=np.float32)[None, None, :]
    return acc
